# revision 17
# baseline (speedup 1.0000x reference)
"""GAT-style attention conv (nn_GatConv_35192962024014) on 8 NeuronCores via Bass.

Math per batch (reference-equivalent):
  e_k[i,j] = sum_d h[i,d] h[j,d] A[k,d]       (symmetric in i,j)
  alpha    = leaky_relu(select_by_adj(e), 0.2), masked -inf where adj==0
  out      = softmax(alpha, -1) @ h

Device mapping (per core, 32 batches; N padded 300->384, E padded 100->128):
  - h uploaded f16; adj uploaded nibble-packed int8 (two 0..4 codes/byte)
  - e'_k = (h*A_k/8192) @ h^T computed f16 x f16 -> fp32 PSUM (A scaled on host)
  - select via  cand_k = (adj != k+1) - e'_k ;  nacc = min_k cand_k
    (selected payload survives exactly; unselected/masked land near +1)
  - lrelu via  m2 = min(nacc, 0.2*nacc);  alpha = -8192*m2
  - exact softmax: rowmax = -8192*min_j(m2);  t = Exp(-8192*m2 - rowmax) in f16,
    denominators via the activation's accum_out; division folded into the
    PSUM drain of the output matmul.
  - out = (t @ h) * (1/denom)  computed as 9 f16 matmuls per batch using
    DMA-xbar transposes of t.
Output is row-quantized on device (int8 values + f32 row scale, packed in one
byte blob) to halve the slow device->host transfer, and dequantized on host.
"""
import numpy as np
import jax

import concourse.bacc as bacc
import concourse.tile as tile
from concourse import mybir
from concourse.bass2jax import _bass_exec_p, install_neuronx_cc_hook, partition_id_tensor
from jax.experimental.shard_map import shard_map
from jax.sharding import Mesh, PartitionSpec

NC = 8
B, N, E = 256, 300, 100
BPC = B // NC          # 32 batches per core
NP = 384               # padded j-range (3 chunks of 128)
EP = 128               # padded contraction dim
CHUNKS = [(0, 128), (128, 256), (256, 384)]  # i/j chunk ranges (padded)
F16 = mybir.dt.float16
F32 = mybir.dt.float32
I8 = mybir.dt.int8


HB = BPC * N * E * 2       # f16 h section bytes
AB = BPC * N * (N // 2)    # nibble-packed adj section bytes
CB = EP * 4 * 4            # acols f32 section bytes
CORE_BYTES = HB + AB + CB
OQB = BPC * N * E          # int8 quantized output section bytes
OSB = BPC * N * 2          # f16 per-row scale section bytes


def _build_program():
    nc = bacc.Bacc("TRN2", target_bir_lowering=False, debug=False)
    # One packed byte-blob input per core: 8 shard transfers instead of 24
    # (each per-shard RPC costs ~10ms of tunnel latency).
    blob = nc.dram_tensor("blob", [CORE_BYTES], I8, kind="ExternalInput").ap()
    h_in = blob[0:HB].bitcast(F16).rearrange("(b n e) -> b n e", n=N, e=E)
    adjp_in = blob[HB:HB + AB].rearrange("(b n j) -> b n j", n=N, j=N // 2)
    ac_in = blob[HB + AB:CORE_BYTES].bitcast(F32).rearrange("(p k) -> p k", k=4)
    # One packed byte-blob output: int8 row-quantized values + f32 row scales.
    ob = nc.dram_tensor("oblob", [OQB + OSB], I8, kind="ExternalOutput").ap()
    q_d = ob[0:OQB].rearrange("(b n e) -> b n e", n=N, e=E)
    sc_d = ob[OQB:OQB + OSB].bitcast(F16).rearrange("(b n one) -> b n one", n=N, one=1)

    with tile.TileContext(nc) as tc:
        with (
            tc.tile_pool(name="const", bufs=1) as constp,
            tc.tile_pool(name="hp", bufs=2) as hp,
            tc.tile_pool(name="htp", bufs=2) as htp,
            tc.tile_pool(name="hap", bufs=2) as hap,
            tc.tile_pool(name="adjp", bufs=2) as adjp,
            tc.tile_pool(name="candp", bufs=2) as candp,
            tc.tile_pool(name="tp", bufs=2) as tpool,
            tc.tile_pool(name="ttp", bufs=2) as ttp,
            tc.tile_pool(name="smallp", bufs=3) as smallp,
            tc.tile_pool(name="outp", bufs=3) as outp,
            tc.tile_pool(name="epsum", bufs=1, space="PSUM") as epsum,
            tc.tile_pool(name="opsum", bufs=1, space="PSUM") as opsum,
        ):
            acols = constp.tile([EP, 4], F32)
            nc.sync.dma_start(acols[:], ac_in)

            for b in range(BPC):
                # ---- load h (f16) into 3 zero-padded [128, EP] chunks ----
                htiles = []
                for ci, (lo, hi) in enumerate(CHUNKS):
                    ht = hp.tile([128, EP], F16, tag=f"h{ci}")
                    nc.gpsimd.memset(ht[:], 0.0)
                    cnt = min(hi, N) - lo
                    nc.sync.dma_start(ht[0:cnt, 0:E], h_in[b, lo:lo + cnt, :])
                    htiles.append(ht)

                # ---- hT [EP, NP] via DMA xbar transpose of each chunk ----
                hT = htp.tile([EP, NP], F16)
                for ci, (lo, hi) in enumerate(CHUNKS):
                    nc.sync.dma_start_transpose(hT[:, lo:hi], htiles[ci][:])

                # ---- rhs_k = hT * (A_k/8192) broadcast per-partition ----
                rhs_k = []
                for k in range(4):
                    r = hap.tile([EP, NP], F16, tag=f"ha{k}")
                    nc.vector.tensor_scalar(r[:], hT[:], acols[:, k:k + 1], None,
                                            mybir.AluOpType.mult)
                    rhs_k.append(r)

                # ---- per i-chunk: e'_k matmuls, select, softmax -> t ----
                ttiles = []
                recips = []
                for ci, (lo, hi) in enumerate(CHUNKS):
                    # adj chunk: memset pads, then nibble-unpack into [128, NP]
                    adjt = adjp.tile([128, NP], I8, tag=f"adj{ci}")
                    nc.gpsimd.memset(adjt[:], 0)
                    apk = adjp.tile([128, N // 2], I8, tag=f"apk{ci}")
                    cnt = min(hi, N) - lo
                    nc.sync.dma_start(apk[0:cnt, :], adjp_in[b, lo:lo + cnt, :])
                    av = adjt[:].rearrange("p (j two) -> p j two", two=2)
                    nc.vector.tensor_scalar(av[:, 0:N // 2, 0], apk[:], 15, None,
                                            mybir.AluOpType.bitwise_and)
                    nc.vector.tensor_scalar(av[:, 0:N // 2, 1], apk[:], 4, None,
                                            mybir.AluOpType.logical_shift_right)

                    # 4 matmuls: weights = hT[:, chunk] ; moving = rhs_k
                    eps = []
                    for k in range(4):
                        ep = epsum.tile([128, NP], F32, tag=f"e{k}")
                        nc.tensor.matmul(ep[:], hT[:, lo:hi], rhs_k[k][:],
                                         start=True, stop=True)
                        eps.append(ep)

                    # cand_k = (adj != k+1) - e'_k ; nacc = min_k cand_k
                    cands = []
                    for k in range(4):
                        c = candp.tile([128, NP], F32, tag=f"c{k}")
                        nc.vector.scalar_tensor_tensor(
                            c[:], adjt[:], float(k + 1), eps[k][:],
                            mybir.AluOpType.not_equal, mybir.AluOpType.subtract)
                        cands.append(c)
                    c01 = candp.tile([128, NP], F32, tag="c01")
                    nc.vector.tensor_tensor(c01[:], cands[0][:], cands[1][:],
                                            mybir.AluOpType.min)
                    c23 = candp.tile([128, NP], F32, tag="c23")
                    nc.vector.tensor_tensor(c23[:], cands[2][:], cands[3][:],
                                            mybir.AluOpType.min)
                    m2 = candp.tile([128, NP], F32, tag="m2")
                    nc.vector.tensor_tensor(m2[:], c01[:], c23[:], mybir.AluOpType.min)
                    # lrelu fold: m2 <- min(nacc, 0.2*nacc)
                    nc.vector.scalar_tensor_tensor(m2[:], m2[:], 0.2, m2[:],
                                                   mybir.AluOpType.mult,
                                                   mybir.AluOpType.min)
                    # exact rowmax: bias = 8192 * min_j m2   (= -rowmax(alpha))
                    mn = smallp.tile([128, 1], F32, tag="mn")
                    nc.vector.tensor_reduce(mn[:], m2[:], mybir.AxisListType.X,
                                            mybir.AluOpType.min)
                    bias = smallp.tile([128, 1], F32, tag="bias")
                    nc.vector.tensor_scalar(bias[:], mn[:], 8192.0, None,
                                            mybir.AluOpType.mult)
                    # t = exp(-8192*m2 + bias) in f16; accum_out = row sums
                    tt = tpool.tile([128, NP], F16, tag=f"t{ci}")
                    den = smallp.tile([128, 1], F32, tag="den")
                    nc.scalar.activation(tt[:], m2[:], mybir.ActivationFunctionType.Exp,
                                         bias=bias[:], scale=-8192.0, accum_out=den[:])
                    rc = smallp.tile([128, 1], F32, tag="rc")
                    nc.vector.reciprocal(rc[:], den[:])
                    ttiles.append(tt)
                    recips.append(rc)

                # ---- tT tiles via DMA xbar transpose ----
                tTs = []
                for cj, (jlo, jhi) in enumerate(CHUNKS):
                    tT = ttp.tile([128, NP], F16, tag=f"tT{cj}")
                    for ci, (ilo, ihi) in enumerate(CHUNKS):
                        nc.sync.dma_start_transpose(tT[:, ilo:ihi],
                                                    ttiles[ci][:, jlo:jhi])
                    tTs.append(tT)

                # ---- out[i,:] = sum_j t[i,j] h[j,:], row-quantized to int8 ----
                # q = round(po * 127/maxabs(po));  scale = maxabs * (1/denom) / 127
                # (the softmax denominator cancels out of q itself)
                for ci, (ilo, ihi) in enumerate(CHUNKS):
                    po = opsum.tile([128, E], F32, tag=f"po{ci}")
                    for cj in range(3):
                        nc.tensor.matmul(po[:], tTs[cj][:, ilo:ihi],
                                         htiles[cj][:, 0:E],
                                         start=(cj == 0), stop=(cj == 2))
                    mab = smallp.tile([128, 1], F32, tag="mab")
                    nc.vector.tensor_reduce(mab[:], po[:], mybir.AxisListType.X,
                                            mybir.AluOpType.max,
                                            apply_absolute_value=True)
                    rab = smallp.tile([128, 1], F32, tag="rab")
                    nc.vector.reciprocal(rab[:], mab[:])
                    q = outp.tile([128, E], I8, tag=f"q{ci}")
                    nc.vector.tensor_scalar(q[:], po[:], rab[:], 127.0,
                                            mybir.AluOpType.mult,
                                            mybir.AluOpType.mult)
                    sc = outp.tile([128, 1], F16, tag=f"sc{ci}")
                    nc.vector.tensor_scalar(sc[:], mab[:], recips[ci][:], 1.0 / 127.0,
                                            mybir.AluOpType.mult,
                                            mybir.AluOpType.mult)
                    cnt = min(ihi, N) - ilo
                    nc.sync.dma_start(q_d[b, ilo:ilo + cnt, :], q[0:cnt, :])
                    nc.sync.dma_start(sc_d[b, ilo:ilo + cnt, :], sc[0:cnt, :])
    nc.compile()
    return nc


class _Runner:
    def __init__(self):
        install_neuronx_cc_hook()
        nc = _build_program()
        self.nc = nc
        partition_name = nc.partition_id_tensor.name if nc.partition_id_tensor else None
        in_names, out_names, out_avals, self.zero_shapes = [], [], [], []
        for alloc in nc.m.functions[0].allocations:
            if not isinstance(alloc, mybir.MemoryLocationSet):
                continue
            name = alloc.memorylocations[0].name
            if alloc.kind == "ExternalInput":
                if name != partition_name:
                    in_names.append(name)
            elif alloc.kind == "ExternalOutput":
                shape = tuple(alloc.tensor_shape)
                dtype = mybir.dt.np(alloc.dtype)
                out_names.append(name)
                out_avals.append(jax.core.ShapedArray(shape, dtype))
                self.zero_shapes.append((shape, dtype))
        self.in_names = in_names
        n_params, n_outs = len(in_names), len(out_names)
        all_names = in_names + out_names + ([partition_name] if partition_name else [])

        def _body(*args):
            operands = list(args)
            if partition_name is not None:
                operands.append(partition_id_tensor())
            outs = _bass_exec_p.bind(
                *operands,
                out_avals=tuple(out_avals),
                in_names=tuple(all_names),
                out_names=tuple(out_names),
                lowering_input_output_aliases=(),
                sim_require_finite=True,
                sim_require_nnan=True,
                nc=nc,
            )
            return tuple(outs)

        devices = jax.devices()[:NC]
        mesh = Mesh(np.asarray(devices), ("core",))
        self.sharded = jax.jit(
            shard_map(_body, mesh=mesh,
                      in_specs=(PartitionSpec("core"),) * (n_params + n_outs),
                      out_specs=(PartitionSpec("core"),) * n_outs,
                      check_rep=False),
            keep_unused=True,
        )
        # The zero "output-init" operands are required by the custom-call
        # signature but the kernel writes every output element, so keep ONE
        # device-resident copy and reuse it (no donation) — saves re-uploading
        # output-sized zeros on every call.
        from jax.sharding import NamedSharding
        zsh = NamedSharding(mesh, PartitionSpec("core"))
        self.dev_zeros = [
            jax.device_put(np.zeros((NC * s[0],) + s[1:], d), zsh)
            for s, d in self.zero_shapes
        ]

    def __call__(self, per_input: dict[str, np.ndarray]) -> np.ndarray:
        ins = [per_input[n] for n in self.in_names]
        outs = self.sharded(*ins, *self.dev_zeros)
        return np.asarray(outs[0])


_RUNNER = None


def _fallback(item_embeddings, adj, A):
    """jax.pmap reference path — only used if the Bass path fails."""
    import jax.numpy as jnp

    def _per_core(h, a, Af):
        hA = h[None] * Af[:, None, None, :]
        e = jnp.einsum('kbid,bjd->kbij', hA, h)
        e = jnp.where(e > 0, e, 0.2 * e)
        alpha = jnp.full(a.shape, -9e15, dtype=jnp.float32)
        for k in range(4):
            alpha = jnp.where(a == k + 1, e[k], alpha)
        alpha = jax.nn.softmax(alpha, axis=-1)
        return jnp.matmul(alpha, h)

    pm = jax.pmap(_per_core, in_axes=(0, 0, None))
    h = np.asarray(item_embeddings, np.float32).reshape(NC, BPC, N, E)
    a8 = np.asarray(adj).astype(np.int8).reshape(NC, BPC, N, N)
    out = pm(h, a8, np.asarray(A, np.float32))
    return np.asarray(jax.device_get(out)).reshape(B, N, E).astype(np.float32)


def kernel(item_embeddings: np.ndarray, adj: np.ndarray, A: np.ndarray) -> np.ndarray:
    global _RUNNER
    if _RUNNER is None:
        try:
            _RUNNER = _Runner()
        except Exception:
            _RUNNER = "fallback"
    if _RUNNER == "fallback":
        return _fallback(item_embeddings, adj, A)

    blob = np.empty((NC, CORE_BYTES), np.int8)
    # cast-copy h straight into the blob's f16 section (no intermediate array);
    # the sliced view keeps the last axis contiguous so .view(f16) is legal
    np.copyto(blob[:, 0:HB].view(np.float16),
              np.asarray(item_embeddings).reshape(NC, BPC * N * E),
              casting="same_kind")
    a8 = np.asarray(adj).astype(np.int8)
    packed = a8[..., 0::2] | (a8[..., 1::2] << 4)
    blob[:, HB:HB + AB] = packed.reshape(NC, AB)
    acols = np.zeros((EP, 4), np.float32)
    acols[:E, :] = (np.asarray(A, np.float32) / 8192.0).T
    blob[:, HB + AB:] = acols.reshape(-1).view(np.int8)[None, :]

    ob = _RUNNER({"blob": blob.reshape(-1)}).reshape(NC, OQB + OSB)
    q = ob[:, 0:OQB].reshape(B * N, E)
    sc = ob[:, OQB:].view(np.float16).reshape(B * N, 1)
    return np.multiply(q, sc, dtype=np.float32).reshape(B, N, E)


# revision 18
# speedup vs baseline: 1.3787x; 1.3787x over previous
"""GAT-style attention conv (nn_GatConv_35192962024014) on 8 NeuronCores via Bass.

Math per batch (reference-equivalent):
  e_k[i,j] = sum_d h[i,d] h[j,d] A[k,d]       (symmetric in i,j)
  alpha    = leaky_relu(select_by_adj(e), 0.2), masked -inf where adj==0
  out      = softmax(alpha, -1) @ h

Device mapping (per core, 32 batches; N padded 300->384, E padded 100->128):
  - h uploaded f16; adj uploaded nibble-packed int8 (two 0..4 codes/byte)
  - e'_k = (h*A_k/8192) @ h^T computed f16 x f16 -> fp32 PSUM (A scaled on host)
  - select via  cand_k = (adj != k+1) - e'_k ;  nacc = min_k cand_k
    (selected payload survives exactly; unselected/masked land near +1)
  - lrelu via  m2 = min(nacc, 0.2*nacc);  alpha = -8192*m2
  - exact softmax: rowmax = -8192*min_j(m2);  t = Exp(-8192*m2 - rowmax) in f16,
    denominators via the activation's accum_out; division folded into the
    PSUM drain of the output matmul.
  - out = (t @ h) * (1/denom)  computed as 9 f16 matmuls per batch using
    DMA-xbar transposes of t.
Output is row-quantized on device (int8 values + f16 row scale, packed in one
byte blob) to halve the slow device->host transfer, and dequantized on host.
"""
import numpy as np
import jax

import concourse.bacc as bacc
import concourse.tile as tile
from concourse import mybir
from concourse.bass2jax import _bass_exec_p, install_neuronx_cc_hook, partition_id_tensor
from jax.experimental.shard_map import shard_map
from jax.sharding import Mesh, PartitionSpec

NC = 8
B, N, E = 256, 300, 100
BPC = B // NC          # 32 batches per core
NP = 384               # padded j-range (3 chunks of 128)
EP = 128               # padded contraction dim
CHUNKS = [(0, 128), (128, 256), (256, 384)]  # i/j chunk ranges (padded)
F16 = mybir.dt.float16
F32 = mybir.dt.float32
I8 = mybir.dt.int8


HB = BPC * N * E * 2       # f16 h section bytes
AB = BPC * N * (N // 2)    # nibble-packed adj section bytes
CB = EP * 4 * 4            # acols f32 section bytes
CORE_BYTES = HB + AB + CB
OQB = BPC * N * E          # int8 quantized output section bytes
OSB = BPC * N * 2          # f16 per-row scale section bytes


def _build_program():
    nc = bacc.Bacc("TRN2", target_bir_lowering=False, debug=False)
    # One packed byte-blob input per core: 8 shard transfers instead of 24
    # (each per-shard RPC costs ~10ms of tunnel latency).
    blob = nc.dram_tensor("blob", [CORE_BYTES], I8, kind="ExternalInput").ap()
    h_in = blob[0:HB].bitcast(F16).rearrange("(b n e) -> b n e", n=N, e=E)
    adjp_in = blob[HB:HB + AB].rearrange("(b n j) -> b n j", n=N, j=N // 2)
    ac_in = blob[HB + AB:CORE_BYTES].bitcast(F32).rearrange("(p k) -> p k", k=4)
    # One packed byte-blob output: int8 row-quantized values + f32 row scales.
    ob = nc.dram_tensor("oblob", [OQB + OSB], I8, kind="ExternalOutput").ap()
    q_d = ob[0:OQB].rearrange("(b n e) -> b n e", n=N, e=E)
    sc_d = ob[OQB:OQB + OSB].bitcast(F16).rearrange("(b n one) -> b n one", n=N, one=1)

    with tile.TileContext(nc) as tc:
        with (
            tc.tile_pool(name="const", bufs=1) as constp,
            tc.tile_pool(name="hp", bufs=2) as hp,
            tc.tile_pool(name="htp", bufs=2) as htp,
            tc.tile_pool(name="hap", bufs=2) as hap,
            tc.tile_pool(name="adjp", bufs=2) as adjp,
            tc.tile_pool(name="candp", bufs=2) as candp,
            tc.tile_pool(name="tp", bufs=2) as tpool,
            tc.tile_pool(name="ttp", bufs=2) as ttp,
            tc.tile_pool(name="smallp", bufs=3) as smallp,
            tc.tile_pool(name="outp", bufs=3) as outp,
            tc.tile_pool(name="epsum", bufs=1, space="PSUM") as epsum,
            tc.tile_pool(name="opsum", bufs=1, space="PSUM") as opsum,
        ):
            acols = constp.tile([EP, 4], F32)
            nc.sync.dma_start(acols[:], ac_in)

            for b in range(BPC):
                # ---- load h (f16) into 3 zero-padded [128, EP] chunks ----
                htiles = []
                for ci, (lo, hi) in enumerate(CHUNKS):
                    ht = hp.tile([128, EP], F16, tag=f"h{ci}")
                    nc.gpsimd.memset(ht[:], 0.0)
                    cnt = min(hi, N) - lo
                    nc.sync.dma_start(ht[0:cnt, 0:E], h_in[b, lo:lo + cnt, :])
                    htiles.append(ht)

                # ---- hT [EP, NP] via DMA xbar transpose of each chunk ----
                hT = htp.tile([EP, NP], F16)
                for ci, (lo, hi) in enumerate(CHUNKS):
                    nc.sync.dma_start_transpose(hT[:, lo:hi], htiles[ci][:])

                # ---- rhs_k = hT * (A_k/8192) broadcast per-partition ----
                rhs_k = []
                for k in range(4):
                    r = hap.tile([EP, NP], F16, tag=f"ha{k}")
                    nc.vector.tensor_scalar(r[:], hT[:], acols[:, k:k + 1], None,
                                            mybir.AluOpType.mult)
                    rhs_k.append(r)

                # ---- per i-chunk: e'_k matmuls, select, softmax -> t ----
                ttiles = []
                recips = []
                for ci, (lo, hi) in enumerate(CHUNKS):
                    # adj chunk: memset pads, then nibble-unpack into [128, NP]
                    adjt = adjp.tile([128, NP], I8, tag=f"adj{ci}")
                    nc.gpsimd.memset(adjt[:], 0)
                    apk = adjp.tile([128, N // 2], I8, tag=f"apk{ci}")
                    cnt = min(hi, N) - lo
                    nc.sync.dma_start(apk[0:cnt, :], adjp_in[b, lo:lo + cnt, :])
                    av = adjt[:].rearrange("p (j two) -> p j two", two=2)
                    nc.vector.tensor_scalar(av[:, 0:N // 2, 0], apk[:], 15, None,
                                            mybir.AluOpType.bitwise_and)
                    nc.vector.tensor_scalar(av[:, 0:N // 2, 1], apk[:], 4, None,
                                            mybir.AluOpType.logical_shift_right)

                    # 4 matmuls: weights = hT[:, chunk] ; moving = rhs_k
                    eps = []
                    for k in range(4):
                        ep = epsum.tile([128, NP], F32, tag=f"e{k}")
                        nc.tensor.matmul(ep[:], hT[:, lo:hi], rhs_k[k][:],
                                         start=True, stop=True)
                        eps.append(ep)

                    # cand_k = (adj != k+1) - e'_k ; nacc = min_k cand_k
                    cands = []
                    for k in range(4):
                        c = candp.tile([128, NP], F32, tag=f"c{k}")
                        nc.vector.scalar_tensor_tensor(
                            c[:], adjt[:], float(k + 1), eps[k][:],
                            mybir.AluOpType.not_equal, mybir.AluOpType.subtract)
                        cands.append(c)
                    c01 = candp.tile([128, NP], F32, tag="c01")
                    nc.vector.tensor_tensor(c01[:], cands[0][:], cands[1][:],
                                            mybir.AluOpType.min)
                    c23 = candp.tile([128, NP], F32, tag="c23")
                    nc.vector.tensor_tensor(c23[:], cands[2][:], cands[3][:],
                                            mybir.AluOpType.min)
                    m2 = candp.tile([128, NP], F32, tag="m2")
                    nc.vector.tensor_tensor(m2[:], c01[:], c23[:], mybir.AluOpType.min)
                    # lrelu fold: m2 <- min(nacc, 0.2*nacc)
                    nc.vector.scalar_tensor_tensor(m2[:], m2[:], 0.2, m2[:],
                                                   mybir.AluOpType.mult,
                                                   mybir.AluOpType.min)
                    # exact rowmax: bias = 8192 * min_j m2   (= -rowmax(alpha))
                    mn = smallp.tile([128, 1], F32, tag="mn")
                    nc.vector.tensor_reduce(mn[:], m2[:], mybir.AxisListType.X,
                                            mybir.AluOpType.min)
                    bias = smallp.tile([128, 1], F32, tag="bias")
                    nc.vector.tensor_scalar(bias[:], mn[:], 8192.0, None,
                                            mybir.AluOpType.mult)
                    # t = exp(-8192*m2 + bias) in f16; accum_out = row sums
                    tt = tpool.tile([128, NP], F16, tag=f"t{ci}")
                    den = smallp.tile([128, 1], F32, tag="den")
                    nc.scalar.activation(tt[:], m2[:], mybir.ActivationFunctionType.Exp,
                                         bias=bias[:], scale=-8192.0, accum_out=den[:])
                    rc = smallp.tile([128, 1], F32, tag="rc")
                    nc.vector.reciprocal(rc[:], den[:])
                    ttiles.append(tt)
                    recips.append(rc)

                # ---- tT tiles via DMA xbar transpose ----
                tTs = []
                for cj, (jlo, jhi) in enumerate(CHUNKS):
                    tT = ttp.tile([128, NP], F16, tag=f"tT{cj}")
                    for ci, (ilo, ihi) in enumerate(CHUNKS):
                        nc.sync.dma_start_transpose(tT[:, ilo:ihi],
                                                    ttiles[ci][:, jlo:jhi])
                    tTs.append(tT)

                # ---- out[i,:] = sum_j t[i,j] h[j,:], row-quantized to int8 ----
                # q = round(po * 127/maxabs(po));  scale = maxabs * (1/denom) / 127
                # (the softmax denominator cancels out of q itself)
                for ci, (ilo, ihi) in enumerate(CHUNKS):
                    po = opsum.tile([128, E], F32, tag=f"po{ci}")
                    for cj in range(3):
                        nc.tensor.matmul(po[:], tTs[cj][:, ilo:ihi],
                                         htiles[cj][:, 0:E],
                                         start=(cj == 0), stop=(cj == 2))
                    mab = smallp.tile([128, 1], F32, tag="mab")
                    nc.vector.tensor_reduce(mab[:], po[:], mybir.AxisListType.X,
                                            mybir.AluOpType.max,
                                            apply_absolute_value=True)
                    rab = smallp.tile([128, 1], F32, tag="rab")
                    nc.vector.reciprocal(rab[:], mab[:])
                    q = outp.tile([128, E], I8, tag=f"q{ci}")
                    nc.vector.tensor_scalar(q[:], po[:], rab[:], 127.0,
                                            mybir.AluOpType.mult,
                                            mybir.AluOpType.mult)
                    sc = outp.tile([128, 1], F16, tag=f"sc{ci}")
                    nc.vector.tensor_scalar(sc[:], mab[:], recips[ci][:], 1.0 / 127.0,
                                            mybir.AluOpType.mult,
                                            mybir.AluOpType.mult)
                    cnt = min(ihi, N) - ilo
                    nc.sync.dma_start(q_d[b, ilo:ilo + cnt, :], q[0:cnt, :])
                    nc.sync.dma_start(sc_d[b, ilo:ilo + cnt, :], sc[0:cnt, :])
    nc.compile()
    return nc


class _Runner:
    def __init__(self):
        install_neuronx_cc_hook()
        nc = _build_program()
        self.nc = nc
        partition_name = nc.partition_id_tensor.name if nc.partition_id_tensor else None
        in_names, out_names, out_avals, self.zero_shapes = [], [], [], []
        for alloc in nc.m.functions[0].allocations:
            if not isinstance(alloc, mybir.MemoryLocationSet):
                continue
            name = alloc.memorylocations[0].name
            if alloc.kind == "ExternalInput":
                if name != partition_name:
                    in_names.append(name)
            elif alloc.kind == "ExternalOutput":
                shape = tuple(alloc.tensor_shape)
                dtype = mybir.dt.np(alloc.dtype)
                out_names.append(name)
                out_avals.append(jax.core.ShapedArray(shape, dtype))
                self.zero_shapes.append((shape, dtype))
        self.in_names = in_names
        n_params, n_outs = len(in_names), len(out_names)
        all_names = in_names + out_names + ([partition_name] if partition_name else [])

        def _body(*args):
            operands = list(args)
            if partition_name is not None:
                operands.append(partition_id_tensor())
            outs = _bass_exec_p.bind(
                *operands,
                out_avals=tuple(out_avals),
                in_names=tuple(all_names),
                out_names=tuple(out_names),
                lowering_input_output_aliases=(),
                sim_require_finite=True,
                sim_require_nnan=True,
                nc=nc,
            )
            return tuple(outs)

        devices = jax.devices()[:NC]
        mesh = Mesh(np.asarray(devices), ("core",))
        self.sharded = jax.jit(
            shard_map(_body, mesh=mesh,
                      in_specs=(PartitionSpec("core"),) * (n_params + n_outs),
                      out_specs=(PartitionSpec("core"),) * n_outs,
                      check_rep=False),
            keep_unused=True,
        )
        # The zero "output-init" operands are required by the custom-call
        # signature but the kernel writes every output element, so keep ONE
        # device-resident copy and reuse it (no donation) — saves re-uploading
        # output-sized zeros on every call.
        from jax.sharding import NamedSharding
        zsh = NamedSharding(mesh, PartitionSpec("core"))
        self.dev_zeros = [
            jax.device_put(np.zeros((NC * s[0],) + s[1:], d), zsh)
            for s, d in self.zero_shapes
        ]

    def __call__(self, per_input: dict[str, np.ndarray]) -> np.ndarray:
        ins = [per_input[n] for n in self.in_names]
        outs = self.sharded(*ins, *self.dev_zeros)
        return np.asarray(outs[0])


_RUNNER = None


def _fallback(item_embeddings, adj, A):
    """jax.pmap reference path — only used if the Bass path fails."""
    import jax.numpy as jnp

    def _per_core(h, a, Af):
        hA = h[None] * Af[:, None, None, :]
        e = jnp.einsum('kbid,bjd->kbij', hA, h)
        e = jnp.where(e > 0, e, 0.2 * e)
        alpha = jnp.full(a.shape, -9e15, dtype=jnp.float32)
        for k in range(4):
            alpha = jnp.where(a == k + 1, e[k], alpha)
        alpha = jax.nn.softmax(alpha, axis=-1)
        return jnp.matmul(alpha, h)

    pm = jax.pmap(_per_core, in_axes=(0, 0, None))
    h = np.asarray(item_embeddings, np.float32).reshape(NC, BPC, N, E)
    a8 = np.asarray(adj).astype(np.int8).reshape(NC, BPC, N, N)
    out = pm(h, a8, np.asarray(A, np.float32))
    return np.asarray(jax.device_get(out)).reshape(B, N, E).astype(np.float32)


def kernel(item_embeddings: np.ndarray, adj: np.ndarray, A: np.ndarray) -> np.ndarray:
    global _RUNNER
    if _RUNNER is None:
        try:
            _RUNNER = _Runner()
        except Exception:
            _RUNNER = "fallback"
    if _RUNNER == "fallback":
        return _fallback(item_embeddings, adj, A)

    blob = np.empty((NC, CORE_BYTES), np.int8)
    # cast-copy h straight into the blob's f16 section (no intermediate array);
    # the sliced view keeps the last axis contiguous so .view(f16) is legal
    np.copyto(blob[:, 0:HB].view(np.float16),
              np.asarray(item_embeddings).reshape(NC, BPC * N * E),
              casting="same_kind")
    a8 = np.asarray(adj).astype(np.int8)
    packed = a8[..., 0::2] | (a8[..., 1::2] << 4)
    blob[:, HB:HB + AB] = packed.reshape(NC, AB)
    acols = np.zeros((EP, 4), np.float32)
    acols[:E, :] = (np.asarray(A, np.float32) / 8192.0).T
    blob[:, HB + AB:] = acols.reshape(-1).view(np.int8)[None, :]

    ob = _RUNNER({"blob": blob.reshape(-1)}).reshape(NC, OQB + OSB)
    q = ob[:, 0:OQB].reshape(B * N, E)
    sc = ob[:, OQB:].view(np.float16).reshape(B * N, 1)
    return np.multiply(q, sc, dtype=np.float32).reshape(B, N, E)


# revision 20
# speedup vs baseline: 1.4019x; 1.0168x over previous
"""GAT-style attention conv (nn_GatConv_35192962024014) on 8 NeuronCores via Bass.

Math per batch (reference-equivalent):
  e_k[i,j] = sum_d h[i,d] h[j,d] A[k,d]       (symmetric in i,j)
  alpha    = leaky_relu(select_by_adj(e), 0.2), masked -inf where adj==0
  out      = softmax(alpha, -1) @ h

Device mapping (per core, 32 batches; N padded 300->384, E padded 100->128):
  - h uploaded f16; adj uploaded nibble-packed int8 (two 0..4 codes/byte)
  - e'_k = (h*A_k/8192) @ h^T computed f16 x f16 -> fp32 PSUM (A scaled on host)
  - select via  cand_k = (adj != k+1) - e'_k ;  nacc = min_k cand_k
    (selected payload survives exactly; unselected/masked land near +1)
  - lrelu via  m2 = min(nacc, 0.2*nacc);  alpha = -8192*m2
  - exact softmax: rowmax = -8192*min_j(m2);  t = Exp(-8192*m2 - rowmax) in f16,
    denominators via the activation's accum_out; division folded into the
    PSUM drain of the output matmul.
  - out = (t @ h) * (1/denom)  computed as 9 f16 matmuls per batch using
    DMA-xbar transposes of t.
Output is row-quantized on device (int8 values + f16 row scale, packed in one
byte blob) to halve the slow device->host transfer, and dequantized on host.
"""
import numpy as np
import jax

import concourse.bacc as bacc
import concourse.tile as tile
from concourse import mybir
from concourse.bass2jax import _bass_exec_p, install_neuronx_cc_hook, partition_id_tensor
from jax.experimental.shard_map import shard_map
from jax.sharding import Mesh, PartitionSpec

NC = 8
B, N, E = 256, 300, 100
BPC = B // NC          # 32 batches per core
NP = 384               # padded j-range (3 chunks of 128)
EP = 128               # padded contraction dim
CHUNKS = [(0, 128), (128, 256), (256, 384)]  # i/j chunk ranges (padded)
F16 = mybir.dt.float16
F32 = mybir.dt.float32
I8 = mybir.dt.int8


HB = BPC * N * E * 2       # f16 h section bytes
AB = BPC * N * (N // 2)    # nibble-packed adj section bytes
CB = EP * 4 * 4            # acols f32 section bytes
CORE_BYTES = HB + AB + CB
OQB = BPC * N * E          # int8 quantized output section bytes
OSB = BPC * N * 2          # f16 per-row scale section bytes


def _build_program():
    nc = bacc.Bacc("TRN2", target_bir_lowering=False, debug=False)
    # One packed byte-blob input per core: 8 shard transfers instead of 24
    # (each per-shard RPC costs ~10ms of tunnel latency).
    blob = nc.dram_tensor("blob", [CORE_BYTES], I8, kind="ExternalInput").ap()
    h_in = blob[0:HB].bitcast(F16).rearrange("(b n e) -> b n e", n=N, e=E)
    adjp_in = blob[HB:HB + AB].rearrange("(b n j) -> b n j", n=N, j=N // 2)
    ac_in = blob[HB + AB:CORE_BYTES].bitcast(F32).rearrange("(p k) -> p k", k=4)
    # One packed byte-blob output: int8 row-quantized values + f32 row scales.
    ob = nc.dram_tensor("oblob", [OQB + OSB], I8, kind="ExternalOutput").ap()
    q_d = ob[0:OQB].rearrange("(b n e) -> b n e", n=N, e=E)
    sc_d = ob[OQB:OQB + OSB].bitcast(F16).rearrange("(b n one) -> b n one", n=N, one=1)

    with tile.TileContext(nc) as tc:
        with (
            tc.tile_pool(name="const", bufs=1) as constp,
            tc.tile_pool(name="hp", bufs=2) as hp,
            tc.tile_pool(name="htp", bufs=2) as htp,
            tc.tile_pool(name="hap", bufs=2) as hap,
            tc.tile_pool(name="adjp", bufs=2) as adjp,
            tc.tile_pool(name="candp", bufs=2) as candp,
            tc.tile_pool(name="tp", bufs=2) as tpool,
            tc.tile_pool(name="ttp", bufs=2) as ttp,
            tc.tile_pool(name="smallp", bufs=3) as smallp,
            tc.tile_pool(name="outp", bufs=3) as outp,
            tc.tile_pool(name="epsum", bufs=1, space="PSUM") as epsum,
            tc.tile_pool(name="opsum", bufs=1, space="PSUM") as opsum,
        ):
            acols = constp.tile([EP, 4], F32)
            nc.sync.dma_start(acols[:], ac_in)

            for b in range(BPC):
                # ---- load h (f16) into 3 zero-padded [128, EP] chunks ----
                htiles = []
                for ci, (lo, hi) in enumerate(CHUNKS):
                    ht = hp.tile([128, EP], F16, tag=f"h{ci}")
                    nc.gpsimd.memset(ht[:], 0.0)
                    cnt = min(hi, N) - lo
                    nc.sync.dma_start(ht[0:cnt, 0:E], h_in[b, lo:lo + cnt, :])
                    htiles.append(ht)

                # ---- hT [EP, NP] via DMA xbar transpose of each chunk ----
                hT = htp.tile([EP, NP], F16)
                for ci, (lo, hi) in enumerate(CHUNKS):
                    nc.sync.dma_start_transpose(hT[:, lo:hi], htiles[ci][:])

                # ---- rhs_k = hT * (A_k/8192) broadcast per-partition ----
                rhs_k = []
                for k in range(4):
                    r = hap.tile([EP, NP], F16, tag=f"ha{k}")
                    nc.vector.tensor_scalar(r[:], hT[:], acols[:, k:k + 1], None,
                                            mybir.AluOpType.mult)
                    rhs_k.append(r)

                # ---- per i-chunk: e'_k matmuls, select, softmax -> t ----
                ttiles = []
                recips = []
                for ci, (lo, hi) in enumerate(CHUNKS):
                    # adj chunk: memset pads, then nibble-unpack into [128, NP]
                    adjt = adjp.tile([128, NP], I8, tag=f"adj{ci}")
                    nc.gpsimd.memset(adjt[:], 0)
                    apk = adjp.tile([128, N // 2], I8, tag=f"apk{ci}")
                    cnt = min(hi, N) - lo
                    nc.sync.dma_start(apk[0:cnt, :], adjp_in[b, lo:lo + cnt, :])
                    av = adjt[:].rearrange("p (j two) -> p j two", two=2)
                    nc.vector.tensor_scalar(av[:, 0:N // 2, 0], apk[:], 15, None,
                                            mybir.AluOpType.bitwise_and)
                    nc.vector.tensor_scalar(av[:, 0:N // 2, 1], apk[:], 4, None,
                                            mybir.AluOpType.logical_shift_right)

                    # 4 matmuls: weights = hT[:, chunk] ; moving = rhs_k
                    eps = []
                    for k in range(4):
                        ep = epsum.tile([128, NP], F32, tag=f"e{k}")
                        nc.tensor.matmul(ep[:], hT[:, lo:hi], rhs_k[k][:],
                                         start=True, stop=True)
                        eps.append(ep)

                    # cand_k = (adj != k+1) - e'_k ; nacc = min_k cand_k
                    cands = []
                    for k in range(4):
                        c = candp.tile([128, NP], F32, tag=f"c{k}")
                        nc.vector.scalar_tensor_tensor(
                            c[:], adjt[:], float(k + 1), eps[k][:],
                            mybir.AluOpType.not_equal, mybir.AluOpType.subtract)
                        cands.append(c)
                    c01 = candp.tile([128, NP], F32, tag="c01")
                    nc.vector.tensor_tensor(c01[:], cands[0][:], cands[1][:],
                                            mybir.AluOpType.min)
                    c23 = candp.tile([128, NP], F32, tag="c23")
                    nc.vector.tensor_tensor(c23[:], cands[2][:], cands[3][:],
                                            mybir.AluOpType.min)
                    m2 = candp.tile([128, NP], F32, tag="m2")
                    nc.vector.tensor_tensor(m2[:], c01[:], c23[:], mybir.AluOpType.min)
                    # lrelu fold: m2 <- min(nacc, 0.2*nacc)
                    nc.vector.scalar_tensor_tensor(m2[:], m2[:], 0.2, m2[:],
                                                   mybir.AluOpType.mult,
                                                   mybir.AluOpType.min)
                    # exact rowmax: bias = 8192 * min_j m2   (= -rowmax(alpha))
                    mn = smallp.tile([128, 1], F32, tag="mn")
                    nc.vector.tensor_reduce(mn[:], m2[:], mybir.AxisListType.X,
                                            mybir.AluOpType.min)
                    bias = smallp.tile([128, 1], F32, tag="bias")
                    nc.vector.tensor_scalar(bias[:], mn[:], 8192.0, None,
                                            mybir.AluOpType.mult)
                    # t = exp(-8192*m2 + bias) in f16; accum_out = row sums
                    tt = tpool.tile([128, NP], F16, tag=f"t{ci}")
                    den = smallp.tile([128, 1], F32, tag="den")
                    nc.scalar.activation(tt[:], m2[:], mybir.ActivationFunctionType.Exp,
                                         bias=bias[:], scale=-8192.0, accum_out=den[:])
                    rc = smallp.tile([128, 1], F32, tag="rc")
                    nc.vector.reciprocal(rc[:], den[:])
                    ttiles.append(tt)
                    recips.append(rc)

                # ---- tT tiles via DMA xbar transpose ----
                tTs = []
                for cj, (jlo, jhi) in enumerate(CHUNKS):
                    tT = ttp.tile([128, NP], F16, tag=f"tT{cj}")
                    for ci, (ilo, ihi) in enumerate(CHUNKS):
                        nc.sync.dma_start_transpose(tT[:, ilo:ihi],
                                                    ttiles[ci][:, jlo:jhi])
                    tTs.append(tT)

                # ---- out[i,:] = sum_j t[i,j] h[j,:], row-quantized to int8 ----
                # q = round(po * 127/maxabs(po));  scale = maxabs * (1/denom) / 127
                # (the softmax denominator cancels out of q itself)
                for ci, (ilo, ihi) in enumerate(CHUNKS):
                    po = opsum.tile([128, E], F32, tag=f"po{ci}")
                    for cj in range(3):
                        nc.tensor.matmul(po[:], tTs[cj][:, ilo:ihi],
                                         htiles[cj][:, 0:E],
                                         start=(cj == 0), stop=(cj == 2))
                    mab = smallp.tile([128, 1], F32, tag="mab")
                    nc.vector.tensor_reduce(mab[:], po[:], mybir.AxisListType.X,
                                            mybir.AluOpType.max,
                                            apply_absolute_value=True)
                    rab = smallp.tile([128, 1], F32, tag="rab")
                    nc.vector.reciprocal(rab[:], mab[:])
                    q = outp.tile([128, E], I8, tag=f"q{ci}")
                    nc.vector.tensor_scalar(q[:], po[:], rab[:], 127.0,
                                            mybir.AluOpType.mult,
                                            mybir.AluOpType.mult)
                    sc = outp.tile([128, 1], F16, tag=f"sc{ci}")
                    nc.vector.tensor_scalar(sc[:], mab[:], recips[ci][:], 1.0 / 127.0,
                                            mybir.AluOpType.mult,
                                            mybir.AluOpType.mult)
                    cnt = min(ihi, N) - ilo
                    nc.sync.dma_start(q_d[b, ilo:ilo + cnt, :], q[0:cnt, :])
                    nc.sync.dma_start(sc_d[b, ilo:ilo + cnt, :], sc[0:cnt, :])
    nc.compile()
    return nc


class _Runner:
    def __init__(self):
        install_neuronx_cc_hook()
        nc = _build_program()
        self.nc = nc
        partition_name = nc.partition_id_tensor.name if nc.partition_id_tensor else None
        in_names, out_names, out_avals, self.zero_shapes = [], [], [], []
        for alloc in nc.m.functions[0].allocations:
            if not isinstance(alloc, mybir.MemoryLocationSet):
                continue
            name = alloc.memorylocations[0].name
            if alloc.kind == "ExternalInput":
                if name != partition_name:
                    in_names.append(name)
            elif alloc.kind == "ExternalOutput":
                shape = tuple(alloc.tensor_shape)
                dtype = mybir.dt.np(alloc.dtype)
                out_names.append(name)
                out_avals.append(jax.core.ShapedArray(shape, dtype))
                self.zero_shapes.append((shape, dtype))
        self.in_names = in_names
        n_params, n_outs = len(in_names), len(out_names)
        all_names = in_names + out_names + ([partition_name] if partition_name else [])

        def _body(*args):
            operands = list(args)
            if partition_name is not None:
                operands.append(partition_id_tensor())
            outs = _bass_exec_p.bind(
                *operands,
                out_avals=tuple(out_avals),
                in_names=tuple(all_names),
                out_names=tuple(out_names),
                lowering_input_output_aliases=(),
                sim_require_finite=True,
                sim_require_nnan=True,
                nc=nc,
            )
            return tuple(outs)

        devices = jax.devices()[:NC]
        mesh = Mesh(np.asarray(devices), ("core",))
        self.sharded = jax.jit(
            shard_map(_body, mesh=mesh,
                      in_specs=(PartitionSpec("core"),) * (n_params + n_outs),
                      out_specs=(PartitionSpec("core"),) * n_outs,
                      check_rep=False),
            keep_unused=True,
        )
        # The zero "output-init" operands are required by the custom-call
        # signature but the kernel writes every output element, so keep ONE
        # device-resident copy and reuse it (no donation) — saves re-uploading
        # output-sized zeros on every call.
        from jax.sharding import NamedSharding
        zsh = NamedSharding(mesh, PartitionSpec("core"))
        self.dev_zeros = [
            jax.device_put(np.zeros((NC * s[0],) + s[1:], d), zsh)
            for s, d in self.zero_shapes
        ]

    def __call__(self, per_input: dict[str, np.ndarray]) -> np.ndarray:
        ins = [per_input[n] for n in self.in_names]
        outs = self.sharded(*ins, *self.dev_zeros)
        return np.asarray(outs[0])


_RUNNER = None
_POOL = None


def _pool():
    global _POOL
    if _POOL is None:
        import concurrent.futures as cf
        _POOL = cf.ThreadPoolExecutor(NC)
    return _POOL


def _fallback(item_embeddings, adj, A):
    """jax.pmap reference path — only used if the Bass path fails."""
    import jax.numpy as jnp

    def _per_core(h, a, Af):
        hA = h[None] * Af[:, None, None, :]
        e = jnp.einsum('kbid,bjd->kbij', hA, h)
        e = jnp.where(e > 0, e, 0.2 * e)
        alpha = jnp.full(a.shape, -9e15, dtype=jnp.float32)
        for k in range(4):
            alpha = jnp.where(a == k + 1, e[k], alpha)
        alpha = jax.nn.softmax(alpha, axis=-1)
        return jnp.matmul(alpha, h)

    pm = jax.pmap(_per_core, in_axes=(0, 0, None))
    h = np.asarray(item_embeddings, np.float32).reshape(NC, BPC, N, E)
    a8 = np.asarray(adj).astype(np.int8).reshape(NC, BPC, N, N)
    out = pm(h, a8, np.asarray(A, np.float32))
    return np.asarray(jax.device_get(out)).reshape(B, N, E).astype(np.float32)


def kernel(item_embeddings: np.ndarray, adj: np.ndarray, A: np.ndarray) -> np.ndarray:
    global _RUNNER
    if _RUNNER is None:
        try:
            _RUNNER = _Runner()
        except Exception:
            _RUNNER = "fallback"
    if _RUNNER == "fallback":
        return _fallback(item_embeddings, adj, A)

    # numpy releases the GIL on large casts/bitwise ops, so the per-core
    # pack/unpack parallelizes across threads
    blob = np.empty((NC, CORE_BYTES), np.int8)
    hsrc = np.asarray(item_embeddings).reshape(NC, BPC * N * E)
    adjsrc = np.asarray(adj).reshape(NC, BPC, N, N)

    def _prep_core(c):
        # cast-copy h straight into the blob's f16 section; the sliced view
        # keeps the last axis contiguous so .view(f16) is legal
        np.copyto(blob[c, 0:HB].view(np.float16), hsrc[c], casting="same_kind")
        a8 = adjsrc[c].astype(np.int8)
        blob[c, HB:HB + AB] = (a8[..., 0::2] | (a8[..., 1::2] << 4)).reshape(AB)

    list(_pool().map(_prep_core, range(NC)))
    acols = np.zeros((EP, 4), np.float32)
    acols[:E, :] = (np.asarray(A, np.float32) / 8192.0).T
    blob[:, HB + AB:] = acols.reshape(-1).view(np.int8)[None, :]

    ob = _RUNNER({"blob": blob.reshape(-1)}).reshape(NC, OQB + OSB)
    res = np.empty((NC, BPC * N, E), np.float32)

    def _dequant_core(c):
        np.multiply(ob[c, 0:OQB].reshape(BPC * N, E),
                    ob[c, OQB:].view(np.float16).reshape(BPC * N, 1),
                    out=res[c], dtype=np.float32)

    list(_pool().map(_dequant_core, range(NC)))
    return res.reshape(B, N, E)
